# revision 1
# baseline (speedup 1.0000x reference)
"""GCN neighborhood mean-aggregation kernel for Trainium2 (8 NeuronCores).

Data-parallel over the batch of target nodes: the embedding table is
replicated to every core, nodes/neigh_idx are sharded along dim 0.  Each
core gathers its nodes' 33 rows (self + 32 sampled neighbors) via SWDGE
indirect DMA and mean-reduces them on VectorE.
"""

import numpy as np

from concourse import bass, bacc, mybir
import concourse.tile as tile
from concourse.bass_utils import run_bass_kernel_spmd

V, D = 100000, 128
B, K = 50000, 32
KP1 = K + 1  # 33 rows per node: self + neighbors
NCORES = 8
P = 128
NBLK = 49            # node blocks of 128 per core
BLOC = NBLK * P      # 6272 padded nodes per core
BPAD = BLOC * NCORES # 50176 >= B


def _build(nblk: int) -> bass.Bass:
    # idx DRAM layout is partition-major: idx[p, i*KP1 + k] holds the k-th
    # index of node i*128+p — the one-shot preload below is then a single
    # contiguous [128, nblk*KP1] DMA, and each block's offsets are a
    # contiguous per-partition slice of the persistent SBUF buffer.
    nc = bacc.Bacc(None)
    feats = nc.declare_dram_parameter(
        "features", [V, D], mybir.dt.float32, isOutput=False
    )
    idx = nc.declare_dram_parameter(
        "idx", [P, nblk * KP1], mybir.dt.int32, isOutput=False
    )
    out = nc.declare_dram_parameter(
        "out", [nblk * P, D], mybir.dt.float32, isOutput=True
    )

    with tile.TileContext(nc) as tc:
        with (
            tc.tile_pool(name="const", bufs=1) as cpool,
            tc.tile_pool(name="sbuf", bufs=3) as pool,
        ):
            idx_buf = cpool.tile([P, nblk * KP1], mybir.dt.int32)
            nc.sync.dma_start(out=idx_buf[:], in_=idx[:])
            for i in range(nblk):
                # 33 gathers per block: the HW indirect DMA consumes ONE
                # offset per partition per instruction, so gather k fetches
                # feats[idx[p, k]] into partition p's k-th row slot.
                gath = pool.tile([P, KP1 * D], mybir.dt.float32, tag="gath")
                for k in range(KP1):
                    nc.gpsimd.indirect_dma_start(
                        out=gath[:, k * D : (k + 1) * D],
                        out_offset=None,
                        in_=feats[:],
                        in_offset=bass.IndirectOffsetOnAxis(
                            ap=idx_buf[:, i * KP1 + k : i * KP1 + k + 1], axis=0
                        ),
                    )
                # Tree-reduce 33 rows into row block 0: fold row 32 in, then
                # halve 32 -> 16 -> 8 -> 4 -> 2 -> 1.
                nc.vector.tensor_add(
                    out=gath[:, 0:D], in0=gath[:, 0:D], in1=gath[:, 32 * D : 33 * D]
                )
                w = 16 * D
                while w >= D:
                    nc.vector.tensor_add(
                        out=gath[:, 0:w], in0=gath[:, 0:w], in1=gath[:, w : 2 * w]
                    )
                    w //= 2
                ot = pool.tile([P, D], mybir.dt.float32, tag="ot")
                nc.vector.tensor_scalar_mul(ot[:], gath[:, 0:D], 1.0 / KP1)
                nc.sync.dma_start(out=out[i * P : (i + 1) * P, :], in_=ot[:])
    nc.finalize()
    return nc


_CACHE: dict = {}

# test-harness knobs (the grading harness leaves these at defaults)
TRACE = False
LAST_RESULTS = None


def _get_nc() -> bass.Bass:
    if "nc" not in _CACHE:
        _CACHE["nc"] = _build(NBLK)
    return _CACHE["nc"]


def kernel(features, nodes, neigh_idx):
    feats = np.ascontiguousarray(np.asarray(features), dtype=np.float32)
    nodes = np.asarray(nodes)
    neigh = np.asarray(neigh_idx)

    idx_all = np.zeros((BPAD, KP1), dtype=np.int32)
    idx_all[:B, 0] = nodes.astype(np.int32)
    idx_all[:B, 1:] = neigh.astype(np.int32)
    # per-core partition-major layout: [NBLK, P, KP1] -> [P, NBLK*KP1]
    shards = (
        idx_all.reshape(NCORES, NBLK, P, KP1)
        .transpose(0, 2, 1, 3)
        .reshape(NCORES, P, NBLK * KP1)
    )

    nc = _get_nc()
    in_maps = [
        {"features": feats, "idx": np.ascontiguousarray(shards[c])}
        for c in range(NCORES)
    ]
    res = run_bass_kernel_spmd(nc, in_maps, list(range(NCORES)), trace=TRACE)
    global LAST_RESULTS
    LAST_RESULTS = res
    out = np.concatenate([res.results[c]["out"] for c in range(NCORES)], axis=0)
    return out[:B]

